# revision 1
# baseline (speedup 1.0000x reference)
"""Trainium2 Bass kernel for EnhancedTripletLoss (hard-mining triplet loss).

Strategy (8 NeuronCores, SPMD, no collectives):
  * Rows (anchors) are sharded BY CLASS: core c handles all anchors of class c
    (8 classes == 8 cores), padded to a uniform 128-aligned slab height Mc.
  * Columns (candidates) are permuted into 8 contiguous class blocks capped at
    1024 columns (exactly two fp32 PSUM banks, so every matmul N-segment is a
    full 512); the excess columns of large classes live in one shared
    OVERFLOW region whose per-class minima become extra bmins columns.
  * Per core, matmuls compute g[a, j] = -2*<e_a, e_j> + ||e_j||^2 into PSUM,
    with the core's OWN class columns sign-negated in the moving operand.  A
    single free-dim min-reduction per class block then yields:
        own block:    min(-g) = -(max over positives of (d2 - sqa))
        other blocks: min( g) =  (min over that block's negatives of (d2-sqa))
    so one pass over the distance matrix produces both the hardest positive
    (argmax) and hardest negative (argmin) statistics.  ||e_a||^2 (constant
    per row) is folded in after the reduction on the Scalar engine.
  * fp32 matmul on TRN2 runs ~4-5x slower than bf16 (HI/LO decomposition),
    so the contraction uses SPLIT-PRECISION bf16: the -2*e_a operand is
    decomposed into NTERMS bf16 terms, the e_j moving operand keeps its bf16
    hi part, and ||e_j||^2 rides as a K=3 chunk of three bf16 terms (exact
    to ~1e-6).  PSUM accumulates in fp32.  Measured loss error vs the fp32
    reference: ~1e-5 (NTERMS=1) / ~4e-6 (NTERMS=2).
  * The K=3 ||e_j||^2 matmuls sit at distinct 32-row PE strips via
    tile_position so their LDWEIGHTS overlap in-flight matmuls and the
    matmuls themselves run concurrently.
  * Per-block minima come from single Vector-engine tensor_reduce ops
    reading PSUM directly (the DVE is the only engine that can both read
    PSUM and reduce on this hardware; it runs ~saturated).
  * The torch F.pairwise_distance eps=1e-6 in the reference perturbs the
    recomputed distances by a relative ~3e-8 (< fp32 ulp) for this data, so
    distances are taken directly from the Gram-trick d2 values.
  * Validity (>=1 positive and >=1 negative) is computed host-side from the
    class counts; invalid/padding anchors are masked by a 0/1 input.
  * Each core writes per-partition partial sums [128, 2] (loss numerator,
    valid count); the host does the final tiny sum + divide.
"""

import numpy as np
import ml_dtypes

P = 128          # SBUF partitions
D = 256          # embedding dim (fixed by the problem)
NCLS = 8         # number of classes == number of cores
NCORES = 8
MARGIN = 0.3
BIGM = 1.0e30    # block-exclusion additive mask (applied to bmins stats only)
NTERMS = 1       # bf16 terms for the -2*e_a stationary operand (1 or 2)
SQTERMS = 3      # bf16 terms for the ||e_j||^2 channel
WMAIN = 1024     # main block cap: 2 fp32 PSUM banks

BF16 = ml_dtypes.bfloat16


def _layout(counts):
    """Main block widths, overflow classes/widths from class counts."""
    wmain = [max(1, min(int(n), WMAIN)) for n in counts]
    ov = [(c, int(n) - WMAIN) for c, n in enumerate(counts) if n > WMAIN]
    ovw = sum(w for _, w in ov)
    assert ovw <= 512, f"overflow region too wide: {ovw}"
    return tuple(wmain), tuple(ov)


def _build_program(Mc, wmain, ov, nterms=NTERMS):
    import concourse.tile as tile
    from concourse import bacc, mybir

    f32 = mybir.dt.float32
    bf16 = mybir.dt.bfloat16
    AX = mybir.AxisListType.X
    OP = mybir.AluOpType

    Mt = Mc // P
    ovw = sum(w for _, w in ov)
    NB = NCLS + len(ov)
    N = int(sum(wmain)) + ovw
    moffs = np.concatenate([[0], np.cumsum(wmain)]).astype(int)

    nc = bacc.Bacc("TRN2", target_bir_lowering=False, debug=False)

    v0d = [nc.dram_tensor(f"v0b{b}", [P, int(wmain[b])], bf16,
                          kind="ExternalInput") for b in range(NCLS)]
    v1d = [nc.dram_tensor(f"v1b{b}", [P, int(wmain[b])], bf16,
                          kind="ExternalInput") for b in range(NCLS)]
    v2d = [nc.dram_tensor(f"v2b{b}", [SQTERMS, int(wmain[b])], bf16,
                          kind="ExternalInput") for b in range(NCLS)]
    ovw_ = sum(w for _, w in ov)
    if ovw_:
        v0od = nc.dram_tensor("v0ov", [P, ovw_], bf16, kind="ExternalInput")
        v1od = nc.dram_tensor("v1ov", [P, ovw_], bf16, kind="ExternalInput")
        v2od = nc.dram_tensor("v2ov", [SQTERMS, ovw_], bf16, kind="ExternalInput")
    uts = [
        nc.dram_tensor(f"u{k}t{t}", [P, Mc], bf16, kind="ExternalInput")
        for t in range(nterms) for k in range(2)
    ]
    sqa = nc.dram_tensor("sqa", [P, Mt], f32, kind="ExternalInput")
    vld = nc.dram_tensor("valid", [P, Mt], f32, kind="ExternalInput")
    pbig = nc.dram_tensor("posbig", [P, NB], f32, kind="ExternalInput")
    nbig = nc.dram_tensor("negbig", [P, NB], f32, kind="ExternalInput")
    out = nc.dram_tensor("out", [P, 2], f32, kind="ExternalOutput")

    with tile.TileContext(nc) as tc:
        with (
            tc.tile_pool(name="resident", bufs=1) as res,
            tc.tile_pool(name="psum", bufs=3, space="PSUM") as pp,
            tc.tile_pool(name="povf", bufs=2, space="PSUM") as po,
            tc.tile_pool(name="bmins", bufs=3) as bmp,
            tc.tile_pool(name="epi", bufs=12) as epi,
        ):
            # ---- PE warmup ------------------------------------------------
            # dummy matmuls during the DMA fill so the PE's HAM clock-gate
            # reaches 8/8 (2.4 GHz) before the real stream starts.
            wsrc = res.tile([P, 512], bf16, tag="wsrc")
            nc.vector.memset(wsrc[:], 0.0)
            wp = pp.tile([P, WMAIN], f32, tag="pblk", name="warm")
            for _ in range(44):
                nc.tensor.matmul(wp[:, 0:512], wsrc[:, 0:P], wsrc[:, :],
                                 start=True, stop=True)

            # ---- resident loads -------------------------------------------
            dma_engs = [nc.sync, nc.scalar, nc.gpsimd]
            _dma_rr = [0]

            def dma(out_ap, in_ap):
                dma_engs[_dma_rr[0] % 3].dma_start(out=out_ap, in_=in_ap)
                _dma_rr[0] += 1

            utiles = []
            for i, ut in enumerate(uts):
                t = res.tile([P, Mc], bf16, tag=f"ut{i}", name=f"ut{i}")
                dma(t[:], ut[:, :])
                utiles.append(t)
            u2t = res.tile([32 + SQTERMS, Mc], bf16, tag="u2")
            nc.vector.memset(u2t[:], 1.0)

            sqat = res.tile([P, Mt], f32, tag="sqa")
            dma(sqat[:], sqa[:, :])
            vldt = res.tile([P, Mt], f32, tag="valid")
            dma(vldt[:], vld[:, :])
            pbigt = res.tile([P, NB], f32, tag="posbig")
            dma(pbigt[:], pbig[:, :])
            nbigt = res.tile([P, NB], f32, tag="negbig")
            dma(nbigt[:], nbig[:, :])

            # per-block V tiles, spread across three engine DMA queues
            v0ts, v1ts, v2ts = [], [], []
            for b in range(NCLS):
                W = int(wmain[b])
                t0 = res.tile([P, W], bf16, tag=f"v0b{b}", name=f"v0b{b}")
                dma(t0[:], v0d[b][:, :])
                t1 = res.tile([P, W], bf16, tag=f"v1b{b}", name=f"v1b{b}")
                dma(t1[:], v1d[b][:, :])
                t2 = res.tile([32 + SQTERMS, W], bf16, tag=f"v2b{b}",
                              name=f"v2b{b}")
                for rp in (0, 32):
                    dma(t2[rp:rp + SQTERMS, :], v2d[b][:, :])
                v0ts.append(t0)
                v1ts.append(t1)
                v2ts.append(t2)
            if ovw:
                ov0 = res.tile([P, ovw], bf16, tag="ov0")
                dma(ov0[:], v0od[:, :])
                ov1 = res.tile([P, ovw], bf16, tag="ov1")
                dma(ov1[:], v1od[:, :])
                ov2 = res.tile([SQTERMS, ovw], bf16, tag="ov2")
                dma(ov2[:], v2od[:, :])

            num_sb = res.tile([P, Mt], f32, tag="num")
            pdists = res.tile([P, Mt], f32, tag="pdists")
            ndists = res.tile([P, Mt], f32, tag="ndists")
            out_sb = res.tile([P, 2], f32, tag="out")

            # ---- main loop ------------------------------------------------
            for mt in range(Mt):
                ms = slice(mt * P, (mt + 1) * P)
                bmins = bmp.tile([P, NB], f32, tag="bm")
                for b in range(NCLS):
                    W = int(wmain[b])
                    ptile = pp.tile([P, W], f32, tag="pblk", name="pblk")
                    segs = [(i, min(512, W - i)) for i in range(0, W, 512)]
                    stats = []
                    for t in range(nterms):
                        stats.append((utiles[2 * t], v0ts[b]))
                        stats.append((utiles[2 * t + 1], v1ts[b]))
                    for ti, (ut, vt) in enumerate(stats):
                        for i, s in segs:
                            cs = slice(i, i + s)
                            nc.tensor.matmul(
                                ptile[:, cs], ut[:, ms], vt[:, cs],
                                start=(ti == 0), stop=False,
                            )
                    for si, (i, s) in enumerate(segs):
                        cs = slice(i, i + s)
                        rp = 32 * (si % 2)
                        nc.tensor.matmul(
                            ptile[:, cs],
                            u2t[rp:rp + SQTERMS, ms],
                            v2ts[b][rp:rp + SQTERMS, cs],
                            start=False, stop=True,
                            tile_position=(rp, 0),
                        )
                    # reduction: min over the block -> bmins[:, b]
                    nc.vector.tensor_reduce(
                        bmins[:, b:b + 1], ptile[:, :], axis=AX, op=OP.min,
                    )

                if ovw:
                    otile = po.tile([P, ovw], f32, tag="ovf", name="ovf")
                    ostats = []
                    for t in range(nterms):
                        ostats.append((utiles[2 * t], ov0))
                        ostats.append((utiles[2 * t + 1], ov1))
                    for ti, (ut, vt) in enumerate(ostats):
                        nc.tensor.matmul(
                            otile[:, :], ut[:, ms], vt[:, :],
                            start=(ti == 0), stop=False,
                        )
                    nc.tensor.matmul(
                        otile[:, :], u2t[0:SQTERMS, ms], ov2[:, :],
                        start=False, stop=True,
                    )
                    oo2 = 0
                    for k, (cls, w) in enumerate(ov):
                        nc.vector.tensor_reduce(
                            bmins[:, NCLS + k:NCLS + k + 1],
                            otile[:, oo2:oo2 + w], axis=AX, op=OP.min,
                        )
                        oo2 += w

                # ---- epilogue for this anchor tile ------------------------
                t8a = epi.tile([P, NB], f32, tag="t8a")
                nc.vector.tensor_add(t8a[:], bmins[:], pbigt[:])
                mown = epi.tile([P, 1], f32, tag="mown")
                nc.vector.tensor_reduce(mown[:], t8a[:], axis=AX, op=OP.min)

                t8b = epi.tile([P, NB], f32, tag="t8b")
                nc.vector.tensor_add(t8b[:], bmins[:], nbigt[:])
                mneg = epi.tile([P, 1], f32, tag="mneg")
                nc.vector.tensor_reduce(mneg[:], t8b[:], axis=AX, op=OP.min)

                # pos_d2 = relu(sqa - m_own), neg_d2 = relu(sqa + m_neg), then
                # sqrt — all on the Scalar engine (fused Relu(scale*x+bias)).
                # Results land in per-mt columns; the final margin/mask math
                # runs ONCE after the loop so the Vector engine's FIFO never
                # blocks on this ACT round-trip between anchor tiles.
                RELU = mybir.ActivationFunctionType.Relu
                pd2c = epi.tile([P, 1], f32, tag="pd2c")
                nc.scalar.activation(pd2c[:], mown[:], RELU,
                                     bias=sqat[:, mt:mt + 1], scale=-1.0)
                nd2c = epi.tile([P, 1], f32, tag="nd2c")
                nc.scalar.activation(nd2c[:], mneg[:], RELU,
                                     bias=sqat[:, mt:mt + 1], scale=1.0)
                nc.scalar.sqrt(pdists[:, mt:mt + 1], pd2c[:])
                nc.scalar.sqrt(ndists[:, mt:mt + 1], nd2c[:])

            # ---- deferred epilogue (one batched pass) ---------------------
            per = epi.tile([P, Mt], f32, tag="per")
            nc.vector.scalar_tensor_tensor(
                per[:], in0=pdists[:], scalar=MARGIN, in1=ndists[:],
                op0=OP.add, op1=OP.subtract,
            )
            perr = epi.tile([P, Mt], f32, tag="perr")
            nc.vector.tensor_scalar_max(perr[:], per[:], 0.0)
            nc.vector.tensor_tensor(num_sb[:], perr[:], vldt[:], op=OP.mult)

            nc.vector.tensor_reduce(out_sb[:, 0:1], num_sb[:], axis=AX, op=OP.add)
            nc.vector.tensor_reduce(out_sb[:, 1:2], vldt[:], axis=AX, op=OP.add)
            nc.sync.dma_start(out=out[:, :], in_=out_sb[:])

    nc.compile()
    return nc


def _bf16_terms(x, nterms):
    """Decompose fp32 array into a list of bf16 terms summing to ~x."""
    terms = []
    r = x.astype(np.float32)
    for _ in range(nterms):
        h = r.astype(BF16)
        terms.append(h)
        r = r - h.astype(np.float32)
    return terms


def _prepare_inputs(emb, lab, nterms=NTERMS):
    """Host-side shard/layout prep.  Returns (in_maps, meta)."""
    B = emb.shape[0]
    assert emb.shape[1] == D
    counts = np.bincount(lab, minlength=NCLS).astype(int)
    assert counts.sum() == B

    order = np.argsort(lab, kind="stable")
    cstart = np.concatenate([[0], np.cumsum(counts)]).astype(int)

    wmain, ov = _layout(counts)
    ovw = sum(w for _, w in ov)
    NB = NCLS + len(ov)
    Mc = int(((max(1, counts.max()) + P - 1) // P) * P)
    Mt = Mc // P
    N = int(sum(wmain)) + ovw

    sq = np.einsum("ij,ij->i", emb, emb, dtype=np.float32)  # ||e||^2, fp32

    # column index: main blocks in class order, then the overflow region
    colidx = np.empty(N, dtype=np.int64)
    own_ranges = {c: [] for c in range(NCLS)}  # column ranges per class
    off = 0
    for c in range(NCLS):
        idx = order[cstart[c]:cstart[c + 1]][:wmain[c]]
        if len(idx) == 0:
            idx = order[0:1]  # arbitrary real point; ties only
        w = wmain[c]
        colidx[off:off + w] = idx
        own_ranges[c].append((off, w))
        off += w
    for cls, w in ov:
        idx = order[cstart[cls] + WMAIN:cstart[cls + 1]]
        assert len(idx) == w
        colidx[off:off + w] = idx
        own_ranges[cls].append((off, w))
        off += w

    Vg = np.ascontiguousarray(emb[colidx].T).astype(BF16)   # [256, N] bf16 hi
    sq_terms = _bf16_terms(sq, SQTERMS)
    sqf_t = np.stack([t[colidx] for t in sq_terms])          # [SQTERMS, N]

    u_full = _bf16_terms(-2.0 * emb, nterms)                 # list of [B, 256]

    # bmins column -> class mapping
    bm_cls = list(range(NCLS)) + [cls for cls, _ in ov]

    in_maps = []
    for c in range(NCLS):
        aidx = order[cstart[c]:cstart[c + 1]]
        if len(aidx) == 0:
            aidx = order[0:1]
        npad = Mc - len(aidx)
        pad = np.full(npad, aidx[0], dtype=np.int64)
        aidx_p = np.concatenate([aidx, pad])

        real = np.zeros(Mc, dtype=np.float32)
        real[: min(len(aidx), Mc)] = 1.0
        cls_valid = 1.0 if (2 <= counts[c] <= B - 1) else 0.0
        valid = (real * cls_valid).reshape(Mt, P).T.copy()  # [128, Mt]

        sqa_t = sq[aidx_p].reshape(Mt, P).T.copy()          # [128, Mt]

        s = np.ones(N, dtype=np.float32)
        for o, w in own_ranges[c]:
            s[o:o + w] = -1.0
        sb = s.astype(BF16)  # +-1 exact

        posbig = np.zeros((P, NB), dtype=np.float32)
        negbig = np.zeros((P, NB), dtype=np.float32)
        for j, bc in enumerate(bm_cls):
            if bc == c:
                negbig[:, j] = BIGM
            else:
                posbig[:, j] = BIGM

        vv0 = Vg[0:128] * sb
        vv1 = Vg[128:256] * sb
        vv2 = sqf_t * sb
        im = {
            "sqa": sqa_t,
            "valid": valid,
            "posbig": posbig,
            "negbig": negbig,
        }
        off2 = 0
        for b in range(NCLS):
            w = wmain[b]
            im[f"v0b{b}"] = np.ascontiguousarray(vv0[:, off2:off2 + w])
            im[f"v1b{b}"] = np.ascontiguousarray(vv1[:, off2:off2 + w])
            im[f"v2b{b}"] = np.ascontiguousarray(vv2[:, off2:off2 + w])
            off2 += w
        if ovw:
            im["v0ov"] = np.ascontiguousarray(vv0[:, off2:])
            im["v1ov"] = np.ascontiguousarray(vv1[:, off2:])
            im["v2ov"] = np.ascontiguousarray(vv2[:, off2:])
        for t in range(nterms):
            ut = u_full[t][aidx_p]                           # [Mc, 256] bf16
            im[f"u0t{t}"] = np.ascontiguousarray(ut[:, 0:128].T)
            im[f"u1t{t}"] = np.ascontiguousarray(ut[:, 128:256].T)
        in_maps.append(im)

    meta = dict(Mc=Mc, wmain=wmain, ov=ov, Mt=Mt, N=N)
    return in_maps, meta


_PROGRAM_CACHE = {}


def _get_program(Mc, wmain, ov):
    key = (Mc, wmain, ov, NTERMS)
    if key not in _PROGRAM_CACHE:
        _PROGRAM_CACHE[key] = _build_program(Mc, wmain, ov, NTERMS)
    return _PROGRAM_CACHE[key]


def _combine(results):
    num = 0.0
    den = 0.0
    for r in results:
        o = np.asarray(r["out"], dtype=np.float64)
        num += o[:, 0].sum()
        den += o[:, 1].sum()
    return np.float32(num / max(den, 1.0))


def _setup_trace_hook():
    """Register the axon NTFF profile hook if the image lacks antenv.axon_hooks."""
    import sys
    import types
    try:
        from antenv.axon_hooks import get_axon_ntff_profile_hook  # noqa: F401
        return
    except ImportError:
        pass
    import antenv
    from trn_agent_boot.trn_boot import _ntff_profile_via_ctypes

    mod = types.ModuleType("antenv.axon_hooks")
    state = {"h": None}
    mod.set_axon_ntff_profile_hook = lambda h: state.__setitem__("h", h)
    mod.get_axon_ntff_profile_hook = lambda: state["h"]
    sys.modules["antenv.axon_hooks"] = mod
    antenv.axon_hooks = mod
    mod.set_axon_ntff_profile_hook(
        _ntff_profile_via_ctypes("/opt/axon/libaxon_pjrt.so")
    )


def kernel(embeddings, labels, _trace=False):
    emb = np.ascontiguousarray(np.asarray(embeddings, dtype=np.float32))
    lab = np.asarray(labels).astype(np.int64).ravel()

    in_maps, meta = _prepare_inputs(emb, lab)
    nc = _get_program(meta["Mc"], meta["wmain"], meta["ov"])

    from concourse.bass_utils import run_bass_kernel_spmd

    if _trace:
        _setup_trace_hook()
        import concourse.bass_utils as _bu
        _bu.upload_artifacts = lambda tmpdir: tmpdir  # skip remote upload

    res = run_bass_kernel_spmd(
        nc, in_maps, core_ids=list(range(NCORES)), trace=bool(_trace),
    )
    loss = _combine(res.results)
    if _trace:
        return loss, res
    return loss

